# revision 48
# baseline (speedup 1.0000x reference)
"""DuoAttention kernel for 8 TRN2 NeuronCores (v2).

Math note: the reference's WINDOW == seq_len, so `local` and `full` are the
same MHA computation. The kernel computes one MHA pass; the duo gate reduces
to a per-batch scalar factor c[i] = (m[i] < 0.1) ? (1 - m[i]) : 1.0 applied
in the broadcast combine out[i, j] = c[i] * mha[j] (shape [B, B, S, D]).

Sharding: data-parallel over batch (2) x tensor-parallel over head groups
(4 groups x 4 heads). Each core computes QKV projections for its 256
features, attention for its 4 heads (2 pairs of 2), and a partial output
projection. The host sums the 4 partials per batch, adds the output bias,
and applies the gate.

v2 design (ScalarE exp is the roofline: 16.8M exps/core ~= 109us + instr
overhead):
  - stream of 256 score blocks [128 keys, 512 queries]; block b = unit
    u=b//2 (section s=u//16, kt=u%16), head-half h=b%2.
  - score MMs row-tiled in concurrent pairs (heads 2p/2p+1 at array rows
    0-63/64-127), K=64 each -> one 512-col slot per pair.
  - PSUM score ring: segments of 3 blocks ([128,1536]) and 2 blocks
    ([128,1024]) alternating -> exp ACT instructions N=1536/1024, 103
    instructions total (vs 128 at N=1024): ScalarE ~139.4us.
  - attn@V pairs col-tiled (M=64 at cols 0/64) into one PSUM bank per
    section, accumulated with start=False over a DVE-zeroed bank.
  - rowsums as 4-way col-tiled M=1 matmuls (rows 0/32/64/96 of one bank),
    one 512-slot per 2 units.
  - projections/outproj drip-fed into PE slack against need-by positions.
"""

import sys

import numpy as np
import ml_dtypes

_REPO = "/opt/trn_rl_repo"
if _REPO not in sys.path:
    sys.path.insert(0, _REPO)

import concourse.bass as bass
import concourse.bacc as bacc
import concourse.mybir as mybir
import concourse.tile as tile
from concourse.bass_utils import run_bass_kernel_spmd

B, S, D, H = 2, 2048, 1024, 16
NCORES = 8
GROUPS = 4            # head groups (tensor parallel)
HPG = H // GROUPS     # 4 heads per group -> 2 pairs
DH = D // H           # 64
GF = HPG * DH         # 256 features per group
DC = D // 128         # 8 contraction chunks of 128
NU = 128              # units: 8 sections x 16 kt
NB = 2 * NU           # score blocks


def trail_pos(u):
    """Stream position at which unit u's attn@V / rowsum work is issued.
    Early units trail with extra lag so the v-projection drip can spread,
    bounded by the exa buffer window."""
    return max(2 * u + 9, u + 18)


TRAIL_AT = {trail_pos(u): u for u in range(1, NU, 2)}  # odd units: pair (u-1,u)
LAST_POS = trail_pos(NU - 1)

BF16 = mybir.dt.bfloat16
F32 = mybir.dt.float32
EXP = mybir.ActivationFunctionType.Exp


def _mk_segs():
    """Score-ring segments: one 2-block segment ([128,1024]) per unit."""
    return [(b, 2) for b in range(0, NB, 2)]


SEGS = _mk_segs()
BLOCK_SEG = {}
for si, (s0, ln) in enumerate(SEGS):
    for b in range(s0, s0 + ln):
        BLOCK_SEG[b] = (si, b - s0)


def build_nc():
    nc = bacc.Bacc("TRN2", target_bir_lowering=False, debug=False,
                   num_devices=NCORES)

    qT = nc.dram_tensor("qT", [D, S], BF16, kind="ExternalInput").ap()
    kT = nc.dram_tensor("kT", [D, S], BF16, kind="ExternalInput").ap()
    vT = nc.dram_tensor("vT", [D, S], BF16, kind="ExternalInput").ap()
    wqT = nc.dram_tensor("wqT", [D, GF], BF16, kind="ExternalInput").ap()
    wkT = nc.dram_tensor("wkT", [D, GF], BF16, kind="ExternalInput").ap()
    wvT = nc.dram_tensor("wvT", [D, GF], BF16, kind="ExternalInput").ap()
    woT = nc.dram_tensor("woT", [GF, D], BF16, kind="ExternalInput").ap()
    bq = nc.dram_tensor("bq", [GF], F32, kind="ExternalInput").ap()
    bk = nc.dram_tensor("bk", [GF], F32, kind="ExternalInput").ap()
    bv = nc.dram_tensor("bv", [GF], BF16, kind="ExternalInput").ap()
    out = nc.dram_tensor("out_part", [S, D], BF16, kind="ExternalOutput").ap()

    # DRAM views: [partition, d-chunk, s-tranche, s-in-tranche]
    qT4 = qT.rearrange("(c p) (t s) -> p c t s", p=128, s=512)
    kT4 = kT.rearrange("(c p) (t s) -> p c t s", p=128, s=512)
    vT4 = vT.rearrange("(c p) (t s) -> p c t s", p=128, s=512)
    wq3 = wqT.rearrange("(c p) f -> p c f", p=128)
    wk3 = wkT.rearrange("(c p) f -> p c f", p=128)
    wv3 = wvT.rearrange("(c p) f -> p c f", p=128)
    wo3 = woT.rearrange("(c p) n -> p c n", p=128)

    with tile.TileContext(nc) as tc:
        with (
            tc.tile_pool(name="const", bufs=1) as const,
            tc.tile_pool(name="acts", bufs=1) as acts,
            tc.tile_pool(name="sc2", bufs=2, space="PSUM") as sc2p,
            tc.tile_pool(name="avp", bufs=2, space="PSUM") as avp,
            tc.tile_pool(name="rsp", bufs=1, space="PSUM") as rsp,
            tc.tile_pool(name="miscp", bufs=1, space="PSUM") as miscp,
            tc.tile_pool(name="exa2", bufs=10) as exa2p,
            tc.tile_pool(name="small", bufs=1) as small,
            tc.tile_pool(name="outsp", bufs=4) as outsp,
        ):
            # ---------------- SBUF persistent tiles ----------------
            wk_sb = const.tile([128, DC, GF], BF16, tag="wk")
            wq_sb = const.tile([128, DC, GF], BF16, tag="wq")
            wv_sb = const.tile([128, DC, GF], BF16, tag="wv")
            wo_sb = const.tile([128, 2, D], BF16, tag="wo")
            bk_sb = const.tile([128, 2], F32, tag="bk")
            bq_sb = const.tile([128, 2], F32, tag="bq")
            bv_sb = const.tile([1, GF], BF16, tag="bv")
            ones_sb = const.tile([128, 1], BF16, tag="ones")
            nc.vector.memset(ones_sb, 1.0)
            onesr_sb = const.tile([1, 128], BF16, tag="onesr")
            nc.vector.memset(onesr_sb, 1.0)

            k_tr = [acts.tile([128, DC, 512], BF16, tag=f"kt{t}",
                              name=f"kt{t}") for t in range(4)]
            q_tr = [acts.tile([128, DC, 512], BF16, tag=f"qt{t}",
                              name=f"qt{t}") for t in range(4)]
            v_tr = [acts.tile([128, DC, 512], BF16, tag=f"v{t}",
                              name=f"v{t}") for t in range(4)]

            kp = [acts.tile([128, S], BF16, tag=f"kp{p}", name=f"kp{p}")
                  for p in range(2)]
            qp = [acts.tile([128, S], BF16, tag=f"qp{p}", name=f"qp{p}")
                  for p in range(2)]
            vp = [acts.tile([128, GF], BF16, tag=f"vp{st}", name=f"vp{st}")
                  for st in range(16)]
            oT = [acts.tile([128, S], BF16, tag=f"oT{p}", name=f"oT{p}")
                  for p in range(2)]

            # ---------------- DMA head (need-ordered) ---------------------
            # One dma_start per tensor/tranche: descriptor generation costs
            # ~650ns of *serial sync-engine time* per call, so fewer+larger
            # calls issue the whole input set fastest; each call internally
            # spreads over all 16 SDMA engines.
            nc.sync.dma_start(out=wk_sb, in_=wk3)
            nc.sync.dma_start(out=k_tr[0], in_=kT4[:, :, 0, :])
            nc.sync.dma_start(out=wq_sb, in_=wq3)
            nc.sync.dma_start(out=q_tr[0], in_=qT4[:, :, 0, :])
            nc.sync.dma_start(out=bk_sb, in_=bk.rearrange("(t p) -> p t", p=128))
            nc.sync.dma_start(out=bq_sb, in_=bq.rearrange("(t p) -> p t", p=128))
            nc.sync.dma_start(out=bv_sb, in_=bv.rearrange("(o f) -> o f", o=1))
            nc.sync.dma_start(out=wv_sb, in_=wv3)
            nc.sync.dma_start(out=v_tr[0], in_=vT4[:, :, 0, :])
            nc.sync.dma_start(out=k_tr[1], in_=kT4[:, :, 1, :])
            nc.sync.dma_start(out=v_tr[1], in_=vT4[:, :, 1, :])
            nc.sync.dma_start(out=q_tr[1], in_=qT4[:, :, 1, :])
            nc.sync.dma_start(out=k_tr[2], in_=kT4[:, :, 2, :])
            nc.sync.dma_start(out=k_tr[3], in_=kT4[:, :, 3, :])
            nc.sync.dma_start(out=v_tr[2], in_=vT4[:, :, 2, :])
            nc.sync.dma_start(out=v_tr[3], in_=vT4[:, :, 3, :])
            nc.sync.dma_start(out=q_tr[2], in_=qT4[:, :, 2, :])
            nc.sync.dma_start(out=q_tr[3], in_=qT4[:, :, 3, :])
            nc.sync.dma_start(out=wo_sb, in_=wo3)

            # ---------------- proj/outproj chain builders -----------------
            def qk_chain_chunks(t, p, st4):
                """k/q projection for (pair p, tranche st4): list of callables."""
                wsb, bsb, dst, xs = ((wq_sb, bq_sb, qp, q_tr) if t == "q"
                                     else (wk_sb, bk_sb, kp, k_tr))
                state = {}

                def mk(dc):
                    def f():
                        if dc == 0:
                            state["ps"] = miscp.tile([128, 512], F32,
                                                     tag="misc", name="ps_qk")
                        nc.tensor.matmul(
                            state["ps"], wsb[:, dc, 128 * p:128 * p + 128],
                            xs[st4][:, dc, :],
                            start=(dc == 0), stop=(dc == DC - 1))
                    return f

                def drain():
                    nc.vector.tensor_scalar_add(
                        dst[p][:, 512 * st4:512 * st4 + 512], state["ps"],
                        bsb[:, p:p + 1])
                return [mk(dc) for dc in range(DC)] + [drain]

            def v_chain_chunks(st, half):
                """v projection for seq chunk st (128 rows), one pair-half
                (128 features): 8 N=128 matmuls + bias + drain (2 chunks)."""
                state = {}
                fs = slice(128 * half, 128 * half + 128)

                def mk(dc0):
                    def f():
                        if dc0 == 0:
                            state["ps"] = miscp.tile([128, 512], F32,
                                                     tag="misc", name="ps_v")
                        ps = state["ps"]
                        for dc in (dc0, dc0 + 1, dc0 + 2):
                            if dc < DC:
                                nc.tensor.matmul(
                                    ps[:, 0:128],
                                    v_tr[st // 4][:, dc, 128 * (st % 4):128 * (st % 4) + 128],
                                    wv_sb[:, dc, fs],
                                    start=(dc == 0), stop=False)
                        if dc0 + 3 > DC:
                            nc.tensor.matmul(ps[:, 0:128], onesr_sb,
                                             bv_sb[0:1, fs],
                                             start=False, stop=True)
                    return f

                def drain():
                    nc.vector.tensor_copy(vp[st][:, fs], state["ps"][:, 0:128])
                return [mk(0), mk(3), mk(6), drain]

            outs_tiles = {}
            outs0 = const.tile([128, 8, 512], BF16, tag="outs0")

            def outproj_chunks(qt, sj, do, ptile=None):
                """One outproj chain: 2 MMs (separate chunks) + copy
                (+ DMA on do==1)."""
                state = {}

                def mm(fc):
                    def f():
                        if fc == 0:
                            if (qt, sj) not in outs_tiles:
                                outs_tiles[(qt, sj)] = outsp.tile(
                                    [128, D], BF16, tag="outs", name="outs")
                            state["ps"] = (ptile if ptile is not None else
                                           miscp.tile([128, 512], F32,
                                                      tag="misc", name="ps_o"))
                        row = 512 * qt + 128 * sj
                        nc.tensor.matmul(
                            state["ps"], oT[fc][:, row:row + 128],
                            wo_sb[:, fc, 512 * do:512 * do + 512],
                            start=(fc == 0), stop=(fc == 1))
                    return f

                def drain():
                    outt = outs_tiles[(qt, sj)]
                    nc.vector.tensor_copy(outt[:, 512 * do:512 * do + 512],
                                          state["ps"])
                    if do == 1:
                        row = 512 * qt + 128 * sj
                        nc.sync.dma_start(out=out[row:row + 128, :], in_=outt)
                        del outs_tiles[(qt, sj)]
                return [mm(0), mm(1), drain]

            def outproj3_pre_chunks(sj, do):
                """qt=3 fc0 partial (available mid-stream, after
                normalize(p0,q3)) -> staged in outs0."""
                state = {}

                def mm():
                    state["ps"] = miscp.tile([128, 512], F32, tag="misc",
                                             name="ps_o3")
                    row = 512 * 3 + 128 * sj
                    nc.tensor.matmul(state["ps"], oT[0][:, row:row + 128],
                                     wo_sb[:, 0, 512 * do:512 * do + 512],
                                     start=True, stop=True)

                def drain():
                    nc.vector.tensor_copy(outs0[:, 2 * sj + do, :],
                                          state["ps"])
                return [mm, drain]

            def outproj3_tail(sj, do, ptile):
                """qt=3 tail: single fc1 MM + add of the staged fc0 partial."""
                state = {}

                def mm():
                    if (3, sj) not in outs_tiles:
                        outs_tiles[(3, sj)] = outsp.tile(
                            [128, D], BF16, tag="outs", name="outs")
                    state["ps"] = ptile
                    row = 512 * 3 + 128 * sj
                    nc.tensor.matmul(state["ps"], oT[1][:, row:row + 128],
                                     wo_sb[:, 1, 512 * do:512 * do + 512],
                                     start=True, stop=True)

                def drain():
                    outt = outs_tiles[(3, sj)]
                    nc.vector.tensor_add(outt[:, 512 * do:512 * do + 512],
                                         state["ps"], outs0[:, 2 * sj + do, :])
                    if do == 1:
                        row = 512 * 3 + 128 * sj
                        nc.sync.dma_start(out=out[row:row + 128, :], in_=outt)
                        del outs_tiles[(3, sj)]
                return [mm, drain]

            # ---------------- drip schedule (greedy backward fill) --------
            # Chains go through ONE serial misc PSUM bank, so they must run
            # contiguously in NEED ORDER (interleaving two chains through the
            # pool would stall the PE queue on the other chain's drain).
            sched = {}

            # prologue chains run before block 0 (positions < 0)
            for i, c in enumerate(qk_chain_chunks("k", 0, 0)):
                sched.setdefault(-10 + i, []).append(c)
            for i, c in enumerate(qk_chain_chunks("q", 0, 0)):
                sched.setdefault(-5 + i, []).append(c)

            chains = []  # (need_by_pos, earliest_pos, chunks)
            for tr in range(1, 4):
                chains.append((8 * tr + 2, 0, qk_chain_chunks("k", 0, tr)))
            for qt in range(1, 4):
                chains.append((32 * qt - 2, 0, qk_chain_chunks("q", 0, qt)))
            # v projections: the p0 halves feed section-0 trail; p1 halves
            # are not needed until the trail crosses into pair-1 (unit 64)
            for st in range(16):
                chains.append((trail_pos(st) - 2, 0, v_chain_chunks(st, 0)))
            for st in range(16):
                chains.append((trail_pos(64 + st) - 4, 0,
                               v_chain_chunks(st, 1)))
            for tr in range(4):
                chains.append((102 + 7 * tr, 0, qk_chain_chunks("k", 1, tr)))
            for qt in range(4):
                chains.append((max(126, 122 + 32 * qt), 0,
                               qk_chain_chunks("q", 1, qt)))
            # outproj qt 0-2: available only after normalize(4+qt)
            for qt in range(3):
                avail = trail_pos(16 * (4 + qt) + 15) + 2
                for sj in range(4):
                    for do in range(2):
                        chains.append((avail + 30, avail,
                                       outproj_chunks(qt, sj, do)))
            # qt=3 fc0 partials: oT[0][:, q3] ready after normalize(p0, q3)
            avail3 = trail_pos(16 * 3 + 15) + 2
            for sj in range(4):
                for do in range(2):
                    chains.append((avail3 + 25, avail3,
                                   outproj3_pre_chunks(sj, do)))

            chains.sort(key=lambda c: c[0])
            cursor = 0
            for need, earliest, chunks in chains:
                start = max(cursor, earliest, need - len(chunks))
                for i, c in enumerate(chunks):
                    sched.setdefault(start + i, []).append(c)
                cursor = start + len(chunks)

            # ---------------- PE warmup (no input deps) -------------------
            # pipelined (rotating psum banks in the sc pool) so it retires in
            # ~2us and never delays the first projection chain
            warm_rhs = const.tile([1, 512], BF16, tag="warm_rhs")
            nc.vector.memset(warm_rhs, 0.0)
            warm_ps = rsp.tile([128, 512], F32, tag="rs", name="warm_ps")
            for j in range(10):
                nc.tensor.matmul(warm_ps[32 * (j % 2):32 * (j % 2) + 1, :],
                                 onesr_sb[0:1, 0:1], warm_rhs,
                                 start=True, stop=True)
            nc.vector.tensor_copy(warm_rhs, warm_ps[0:1, :])

            # ---------------- streaming state -----------------------------
            sc_tiles = {}     # seg -> psum tile
            exa_tiles = {}    # seg -> sbuf tile
            av_t = {}         # section -> av psum tile
            rs_t = {}         # section -> rs psum tile

            def exa_ap(b):
                si, off = BLOCK_SEG[b]
                return exa_tiles[si][:, 512 * off:512 * off + 512]

            def score_mm(b):
                u, h = b // 2, b % 2
                s, kt = u // 16, u % 16
                p, qt = s // 4, s % 4
                si, off = BLOCK_SEG[b]
                if off == 0:
                    sc_tiles[si] = sc2p.tile([128, 1024], F32,
                                             tag="sc2", name="sc")
                sc = sc_tiles[si]
                nc.tensor.matmul(
                    sc[:, 512 * off:512 * off + 512],
                    kp[p][64 * h:64 * h + 64, 128 * kt:128 * kt + 128],
                    qp[p][64 * h:64 * h + 64, 512 * qt:512 * qt + 512],
                    start=True, stop=True)

            def exp_seg(si):
                exa_tiles[si] = exa2p.tile([128, 1024], BF16,
                                           tag="exa2", name="exa")
                nc.scalar.activation(out=exa_tiles[si], in_=sc_tiles[si],
                                     func=EXP, scale=1.0 / np.sqrt(DH))
                del sc_tiles[si]

            def section_start(s):
                av_t[s] = avp.tile([128, 512], F32, tag="av", name="av")
                rs_t[s] = rsp.tile([128, 512], F32, tag="rs", name="rs")
                nc.vector.memset(av_t[s], 0.0)
                nc.vector.memset(rs_t[s], 0.0)

            def trail_pair(u):
                """Trail work for units (u-1, u), u odd: 4 av MMs as two
                col-tiled pairs back-to-back, then the 4-way rowsum quad.
                Same-kind matmuls are batched so the tile-position
                concurrency engages (interleaving kinds serializes them)."""
                s = u // 16
                p = s // 4
                if (u - 1) % 16 == 0:
                    section_start(s)
                av = av_t[s]
                for uu in (u - 1, u):
                    st = uu % 16
                    for h in range(2):
                        nc.tensor.matmul(
                            av[64 * h:64 * h + 64, :],
                            vp[st][:, 128 * p + 64 * h:128 * p + 64 * h + 64],
                            exa_ap(2 * uu + h),
                            start=False, stop=(uu % 16 == 15),
                            tile_position=(0, 64 * h), skip_group_check=True)
                rs = rs_t[s]
                last = (u % 16 == 15)
                if s == 7:
                    # both parities accumulate into rows 0/32 so the tail
                    # normalize needs no cross-partition DMA combine
                    for uu in (u - 1, u):
                        for h in range(2):
                            nc.tensor.matmul(
                                rs[32 * h:32 * h + 1, :], ones_sb,
                                exa_ap(2 * uu + h),
                                start=False, stop=last,
                                tile_position=(0, 32 * h),
                                skip_group_check=True)
                else:
                    for (uu, h) in ((u - 1, 0), (u - 1, 1), (u, 0), (u, 1)):
                        row = 64 * (uu % 2) + 32 * h
                        nc.tensor.matmul(
                            rs[row:row + 1, :], ones_sb, exa_ap(2 * uu + h),
                            start=False, stop=last,
                            tile_position=(0, row), skip_group_check=True)
                if last:
                    normalize(s)

            def normalize(s):
                p, qt = s // 4, s % 4
                av, rs = av_t.pop(s), rs_t.pop(s)
                stage = small.tile([128, 512], F32, tag="stage", name="stage")
                nc.vector.tensor_copy(stage, rs)
                nrm = small.tile([1, 2048], F32, tag="nrm", name="nrm")
                bc = small.tile([128, 1024], F32, tag="bc", name="bc")
                # den layout: [h0_even | h1_even | h0_odd | h1_odd]
                nc.sync.dma_start(out=nrm[0:1, 0:512], in_=stage[0:1, :])
                nc.sync.dma_start(out=nrm[0:1, 512:1024], in_=stage[32:33, :])
                if s != 7:
                    nc.sync.dma_start(out=nrm[0:1, 1024:1536],
                                      in_=stage[64:65, :])
                    nc.sync.dma_start(out=nrm[0:1, 1536:2048],
                                      in_=stage[96:97, :])
                    nc.vector.tensor_add(nrm[0:1, 0:1024], nrm[0:1, 0:1024],
                                         nrm[0:1, 1024:2048])
                rr = nrm[0:1, 1024:2048]
                nc.vector.reciprocal_approx_fast(rr, nrm[0:1, 0:1024])
                nc.gpsimd.partition_broadcast(bc, rr)
                nc.vector.tensor_mul(oT[p][0:64, 512 * qt:512 * qt + 512],
                                     av[0:64, :], bc[0:64, 0:512])
                nc.vector.tensor_mul(oT[p][64:128, 512 * qt:512 * qt + 512],
                                     av[64:128, :], bc[64:128, 512:1024])

            # ---------------- main stream ---------------------------------
            START = min(list(sched.keys()) + [0])
            for pos in range(START, 0):
                for fn in sched.pop(pos, []):
                    fn()

            # score MMs are emitted in 2-unit batches (4 adjacent MMs
            # alternating row halves) so the row-tiled concurrency engages;
            # the two exps follow immediately so ACT is never starved
            for pos in range(LAST_POS + 1):
                if pos < NB and pos % 4 == 0:
                    for b in range(pos, pos + 4):
                        score_mm(b)
                    exp_seg(pos // 2)
                    exp_seg(pos // 2 + 1)
                if pos in TRAIL_AT:
                    trail_pair(TRAIL_AT[pos])
                for fn in sched.pop(pos, []):
                    fn()

            # ---------------- tail: outproj for qt=3 ----------------------
            # rotate psum across the freed score banks for pipelining
            tail_ps = []
            t2a = sc2p.tile([128, 1024], F32, tag="sc2", name="sc_tail")
            t2b = sc2p.tile([128, 1024], F32, tag="sc2", name="sc_tail2")
            for t2 in (t2a, t2b):
                for j in range(2):
                    tail_ps.append(t2[:, 512 * j:512 * j + 512])
            jobs = [(sj, do) for sj in range(4) for do in range(2)]
            pending = []
            for i, (sj, do) in enumerate(jobs):
                mm, drain = outproj3_tail(sj, do, tail_ps[i % len(tail_ps)])
                mm()
                pending.append(drain)
                if len(pending) >= 3:
                    pending.pop(0)()
            for d in pending:
                d()
            # flush any unscheduled chunks (shouldn't happen)
            for pos in sorted(sched):
                for fn in sched[pos]:
                    fn()

    nc.compile()
    return nc


_CACHE = {}


def _get_nc():
    if "nc" not in _CACHE:
        _CACHE["nc"] = build_nc()
    return _CACHE["nc"]


def _prep_inputs(query, key, value, in_proj_w, in_proj_b, out_proj_w):
    bf16 = ml_dtypes.bfloat16
    wq, wk, wv = (in_proj_w[0:D], in_proj_w[D:2 * D], in_proj_w[2 * D:3 * D])
    bq, bk, bv = (in_proj_b[0:D], in_proj_b[D:2 * D], in_proj_b[2 * D:3 * D])

    qT = [np.ascontiguousarray(query[b].T).astype(bf16) for b in range(B)]
    kT = [np.ascontiguousarray(key[b].T).astype(bf16) for b in range(B)]
    vT = [np.ascontiguousarray(value[b].T).astype(bf16) for b in range(B)]

    in_maps = []
    for b in range(B):
        for g in range(GROUPS):
            fs = slice(GF * g, GF * (g + 1))
            in_maps.append({
                "qT": qT[b], "kT": kT[b], "vT": vT[b],
                "wqT": np.ascontiguousarray(wq[fs].T).astype(bf16),
                "wkT": np.ascontiguousarray(wk[fs].T).astype(bf16),
                "wvT": np.ascontiguousarray(wv[fs].T).astype(bf16),
                "woT": np.ascontiguousarray(out_proj_w[:, fs].T).astype(bf16),
                "bq": np.ascontiguousarray(bq[fs]).astype(np.float32),
                "bk": np.ascontiguousarray(bk[fs]).astype(np.float32),
                "bv": np.ascontiguousarray(bv[fs]).astype(bf16),
            })
    return in_maps


def kernel(query, key, value, in_proj_w, in_proj_b, out_proj_w, out_proj_b,
           mask_w, mask_b, _run_kwargs=None):
    query = np.asarray(query, np.float32)
    key = np.asarray(key, np.float32)
    value = np.asarray(value, np.float32)
    in_proj_w = np.asarray(in_proj_w, np.float32)
    in_proj_b = np.asarray(in_proj_b, np.float32)
    out_proj_w = np.asarray(out_proj_w, np.float32)
    out_proj_b = np.asarray(out_proj_b, np.float32)
    mask_w = np.asarray(mask_w, np.float32)
    mask_b = np.asarray(mask_b, np.float32)

    in_maps = _prep_inputs(query, key, value, in_proj_w, in_proj_b, out_proj_w)
    nc = _get_nc()
    for _attempt in range(3):
        res = run_bass_kernel_spmd(nc, in_maps, core_ids=list(range(NCORES)),
                                   **(_run_kwargs or {}))
        parts = [np.asarray(r["out_part"], np.float32) for r in res.results]
        # guard against rare transient device glitches: partial outputs are
        # normally bounded well under 1
        if all(np.isfinite(p).all() and np.abs(p).max() < 100.0 for p in parts):
            break
    mha = np.stack(
        [sum(parts[b * GROUPS + g] for g in range(GROUPS)) for b in range(B)],
        axis=0,
    ) + out_proj_b[None, None, :].astype(np.float32)

    logit = (query[:, -1] @ mask_w.T + mask_b).astype(np.float64)
    m = (1.0 / (1.0 + np.exp(-logit))).astype(np.float32).reshape(B)
    c = np.where(m < 0.1, np.float32(1.0) - m, np.float32(1.0))

    out_full = c[:, None, None, None] * mha[None, :, :, :]
    if _run_kwargs is not None:
        _CACHE["last_results"] = res
    return out_full.astype(np.float32)


# revision 49
# speedup vs baseline: 1.2413x; 1.2413x over previous
"""DuoAttention kernel for 8 TRN2 NeuronCores (v2).

Math note: the reference's WINDOW == seq_len, so `local` and `full` are the
same MHA computation. The kernel computes one MHA pass; the duo gate reduces
to a per-batch scalar factor c[i] = (m[i] < 0.1) ? (1 - m[i]) : 1.0 applied
in the broadcast combine out[i, j] = c[i] * mha[j] (shape [B, B, S, D]).

Sharding: data-parallel over batch (2) x tensor-parallel over head groups
(4 groups x 4 heads). Each core computes QKV projections for its 256
features, attention for its 4 heads (2 pairs of 2), and a partial output
projection. The host sums the 4 partials per batch, adds the output bias,
and applies the gate.

v2 design (ScalarE exp is the roofline: 16.8M exps/core ~= 109us + instr
overhead):
  - stream of 256 score blocks [128 keys, 512 queries]; block b = unit
    u=b//2 (section s=u//16, kt=u%16), head-half h=b%2.
  - score MMs row-tiled in concurrent pairs (heads 2p/2p+1 at array rows
    0-63/64-127), K=64 each -> one 512-col slot per pair.
  - PSUM score ring: segments of 3 blocks ([128,1536]) and 2 blocks
    ([128,1024]) alternating -> exp ACT instructions N=1536/1024, 103
    instructions total (vs 128 at N=1024): ScalarE ~139.4us.
  - attn@V pairs col-tiled (M=64 at cols 0/64) into one PSUM bank per
    section, accumulated with start=False over a DVE-zeroed bank.
  - rowsums as 4-way col-tiled M=1 matmuls (rows 0/32/64/96 of one bank),
    one 512-slot per 2 units.
  - projections/outproj drip-fed into PE slack against need-by positions.
"""

import sys

import numpy as np
import ml_dtypes

_REPO = "/opt/trn_rl_repo"
if _REPO not in sys.path:
    sys.path.insert(0, _REPO)

import concourse.bass as bass
import concourse.bacc as bacc
import concourse.mybir as mybir
import concourse.tile as tile
from concourse.bass_utils import run_bass_kernel_spmd

B, S, D, H = 2, 2048, 1024, 16
NCORES = 8
GROUPS = 4            # head groups (tensor parallel)
HPG = H // GROUPS     # 4 heads per group -> 2 pairs
DH = D // H           # 64
GF = HPG * DH         # 256 features per group
DC = D // 128         # 8 contraction chunks of 128
NU = 128              # units: 8 sections x 16 kt
NB = 2 * NU           # score blocks


def trail_pos(u):
    """Stream position at which unit u's attn@V / rowsum work is issued.
    Early units trail with extra lag so the v-projection drip can spread,
    bounded by the exa buffer window."""
    return max(2 * u + 9, u + 18)


TRAIL_AT = {trail_pos(u): u for u in range(1, NU, 2)}  # odd units: pair (u-1,u)
LAST_POS = trail_pos(NU - 1)

BF16 = mybir.dt.bfloat16
F32 = mybir.dt.float32
EXP = mybir.ActivationFunctionType.Exp


def _mk_segs():
    """Score-ring segments: one 2-block segment ([128,1024]) per unit."""
    return [(b, 2) for b in range(0, NB, 2)]


SEGS = _mk_segs()
BLOCK_SEG = {}
for si, (s0, ln) in enumerate(SEGS):
    for b in range(s0, s0 + ln):
        BLOCK_SEG[b] = (si, b - s0)


def build_nc():
    nc = bacc.Bacc("TRN2", target_bir_lowering=False, debug=False,
                   num_devices=NCORES)

    qT = nc.dram_tensor("qT", [D, S], BF16, kind="ExternalInput").ap()
    kT = nc.dram_tensor("kT", [D, S], BF16, kind="ExternalInput").ap()
    vT = nc.dram_tensor("vT", [D, S], BF16, kind="ExternalInput").ap()
    wqT = nc.dram_tensor("wqT", [D, GF], BF16, kind="ExternalInput").ap()
    wkT = nc.dram_tensor("wkT", [D, GF], BF16, kind="ExternalInput").ap()
    wvT = nc.dram_tensor("wvT", [D, GF], BF16, kind="ExternalInput").ap()
    woT = nc.dram_tensor("woT", [GF, D], BF16, kind="ExternalInput").ap()
    bq = nc.dram_tensor("bq", [GF], F32, kind="ExternalInput").ap()
    bk = nc.dram_tensor("bk", [GF], F32, kind="ExternalInput").ap()
    bv = nc.dram_tensor("bv", [GF], BF16, kind="ExternalInput").ap()
    out = nc.dram_tensor("out_part", [S, D], BF16, kind="ExternalOutput").ap()

    # DRAM views: [partition, d-chunk, s-tranche, s-in-tranche]
    qT4 = qT.rearrange("(c p) (t s) -> p c t s", p=128, s=512)
    kT4 = kT.rearrange("(c p) (t s) -> p c t s", p=128, s=512)
    vT4 = vT.rearrange("(c p) (t s) -> p c t s", p=128, s=512)
    wq3 = wqT.rearrange("(c p) f -> p c f", p=128)
    wk3 = wkT.rearrange("(c p) f -> p c f", p=128)
    wv3 = wvT.rearrange("(c p) f -> p c f", p=128)
    wo3 = woT.rearrange("(c p) n -> p c n", p=128)

    with tile.TileContext(nc) as tc:
        with (
            tc.tile_pool(name="const", bufs=1) as const,
            tc.tile_pool(name="acts", bufs=1) as acts,
            tc.tile_pool(name="sc2", bufs=2, space="PSUM") as sc2p,
            tc.tile_pool(name="avp", bufs=2, space="PSUM") as avp,
            tc.tile_pool(name="rsp", bufs=1, space="PSUM") as rsp,
            tc.tile_pool(name="miscp", bufs=1, space="PSUM") as miscp,
            tc.tile_pool(name="exa2", bufs=10) as exa2p,
            tc.tile_pool(name="small", bufs=1) as small,
            tc.tile_pool(name="outsp", bufs=4) as outsp,
        ):
            # ---------------- SBUF persistent tiles ----------------
            wk_sb = const.tile([128, DC, GF], BF16, tag="wk")
            wq_sb = const.tile([128, DC, GF], BF16, tag="wq")
            wv_sb = const.tile([128, DC, GF], BF16, tag="wv")
            wo_sb = const.tile([128, 2, D], BF16, tag="wo")
            bk_sb = const.tile([128, 2], F32, tag="bk")
            bq_sb = const.tile([128, 2], F32, tag="bq")
            bv_sb = const.tile([1, GF], BF16, tag="bv")
            ones_sb = const.tile([128, 1], BF16, tag="ones")
            nc.vector.memset(ones_sb, 1.0)
            onesr_sb = const.tile([1, 128], BF16, tag="onesr")
            nc.vector.memset(onesr_sb, 1.0)

            k_tr = [acts.tile([128, DC, 512], BF16, tag=f"kt{t}",
                              name=f"kt{t}") for t in range(4)]
            q_tr = [acts.tile([128, DC, 512], BF16, tag=f"qt{t}",
                              name=f"qt{t}") for t in range(4)]
            v_tr = [acts.tile([128, DC, 512], BF16, tag=f"v{t}",
                              name=f"v{t}") for t in range(4)]

            kp = [acts.tile([128, S], BF16, tag=f"kp{p}", name=f"kp{p}")
                  for p in range(2)]
            qp = [acts.tile([128, S], BF16, tag=f"qp{p}", name=f"qp{p}")
                  for p in range(2)]
            vp = [acts.tile([128, GF], BF16, tag=f"vp{st}", name=f"vp{st}")
                  for st in range(16)]
            oT = [acts.tile([128, S], BF16, tag=f"oT{p}", name=f"oT{p}")
                  for p in range(2)]

            # ---------------- DMA head (need-ordered) ---------------------
            # One dma_start per tensor/tranche: descriptor generation costs
            # ~650ns of *serial sync-engine time* per call, so fewer+larger
            # calls issue the whole input set fastest; each call internally
            # spreads over all 16 SDMA engines.
            nc.sync.dma_start(out=wk_sb, in_=wk3)
            nc.sync.dma_start(out=k_tr[0], in_=kT4[:, :, 0, :])
            nc.sync.dma_start(out=wq_sb, in_=wq3)
            nc.sync.dma_start(out=q_tr[0], in_=qT4[:, :, 0, :])
            nc.sync.dma_start(out=bk_sb, in_=bk.rearrange("(t p) -> p t", p=128))
            nc.sync.dma_start(out=bq_sb, in_=bq.rearrange("(t p) -> p t", p=128))
            nc.sync.dma_start(out=bv_sb, in_=bv.rearrange("(o f) -> o f", o=1))
            nc.sync.dma_start(out=wv_sb, in_=wv3)
            nc.sync.dma_start(out=v_tr[0], in_=vT4[:, :, 0, :])
            nc.sync.dma_start(out=k_tr[1], in_=kT4[:, :, 1, :])
            nc.sync.dma_start(out=v_tr[1], in_=vT4[:, :, 1, :])
            nc.sync.dma_start(out=q_tr[1], in_=qT4[:, :, 1, :])
            nc.sync.dma_start(out=k_tr[2], in_=kT4[:, :, 2, :])
            nc.sync.dma_start(out=k_tr[3], in_=kT4[:, :, 3, :])
            nc.sync.dma_start(out=v_tr[2], in_=vT4[:, :, 2, :])
            nc.sync.dma_start(out=v_tr[3], in_=vT4[:, :, 3, :])
            nc.sync.dma_start(out=q_tr[2], in_=qT4[:, :, 2, :])
            nc.sync.dma_start(out=q_tr[3], in_=qT4[:, :, 3, :])
            nc.sync.dma_start(out=wo_sb, in_=wo3)

            # ---------------- proj/outproj chain builders -----------------
            def qk_chain_chunks(t, p, st4):
                """k/q projection for (pair p, tranche st4): list of callables."""
                wsb, bsb, dst, xs = ((wq_sb, bq_sb, qp, q_tr) if t == "q"
                                     else (wk_sb, bk_sb, kp, k_tr))
                state = {}

                def mk(dc0):
                    def f():
                        if dc0 == 0:
                            state["ps"] = miscp.tile([128, 512], F32,
                                                     tag="misc", name="ps_qk")
                        for dc in (dc0, dc0 + 1):
                            nc.tensor.matmul(
                                state["ps"], wsb[:, dc, 128 * p:128 * p + 128],
                                xs[st4][:, dc, :],
                                start=(dc == 0), stop=(dc == DC - 1))
                    return f

                def drain():
                    nc.vector.tensor_scalar_add(
                        dst[p][:, 512 * st4:512 * st4 + 512], state["ps"],
                        bsb[:, p:p + 1])
                return [mk(0), mk(2), mk(4), mk(6), drain]

            def v_chain_chunks(st, half):
                """v projection for seq chunk st (128 rows), one pair-half
                (128 features): 8 N=128 matmuls + bias + drain (2 chunks)."""
                state = {}
                fs = slice(128 * half, 128 * half + 128)

                def mms():
                    state["ps"] = miscp.tile([128, 512], F32,
                                             tag="misc", name="ps_v")
                    ps = state["ps"]
                    for dc in range(DC):
                        nc.tensor.matmul(
                            ps[:, 0:128],
                            v_tr[st // 4][:, dc, 128 * (st % 4):128 * (st % 4) + 128],
                            wv_sb[:, dc, fs],
                            start=(dc == 0), stop=False)
                    nc.tensor.matmul(ps[:, 0:128], onesr_sb, bv_sb[0:1, fs],
                                     start=False, stop=True)

                def drain():
                    nc.vector.tensor_copy(vp[st][:, fs], state["ps"][:, 0:128])
                return [mms, drain]

            outs_tiles = {}
            outs0 = const.tile([128, 8, 512], BF16, tag="outs0")

            def outproj_chunks(qt, sj, do, ptile=None):
                """One outproj chain: 2 MMs (separate chunks) + copy
                (+ DMA on do==1)."""
                state = {}

                def mm(fc):
                    def f():
                        if fc == 0:
                            if (qt, sj) not in outs_tiles:
                                outs_tiles[(qt, sj)] = outsp.tile(
                                    [128, D], BF16, tag="outs", name="outs")
                            state["ps"] = (ptile if ptile is not None else
                                           miscp.tile([128, 512], F32,
                                                      tag="misc", name="ps_o"))
                        row = 512 * qt + 128 * sj
                        nc.tensor.matmul(
                            state["ps"], oT[fc][:, row:row + 128],
                            wo_sb[:, fc, 512 * do:512 * do + 512],
                            start=(fc == 0), stop=(fc == 1))
                    return f

                def drain():
                    outt = outs_tiles[(qt, sj)]
                    nc.vector.tensor_copy(outt[:, 512 * do:512 * do + 512],
                                          state["ps"])
                    if do == 1:
                        row = 512 * qt + 128 * sj
                        nc.sync.dma_start(out=out[row:row + 128, :], in_=outt)
                        del outs_tiles[(qt, sj)]
                def both():
                    mm(0)()
                    mm(1)()
                return [both, drain]

            def outproj3_pre_chunks(sj, do):
                """qt=3 fc0 partial (available mid-stream, after
                normalize(p0,q3)) -> staged in outs0."""
                state = {}

                def mm():
                    state["ps"] = miscp.tile([128, 512], F32, tag="misc",
                                             name="ps_o3")
                    row = 512 * 3 + 128 * sj
                    nc.tensor.matmul(state["ps"], oT[0][:, row:row + 128],
                                     wo_sb[:, 0, 512 * do:512 * do + 512],
                                     start=True, stop=True)

                def drain():
                    nc.vector.tensor_copy(outs0[:, 2 * sj + do, :],
                                          state["ps"])
                return [mm, drain]

            def outproj3_tail(sj, do, ptile):
                """qt=3 tail: single fc1 MM + add of the staged fc0 partial."""
                state = {}

                def mm():
                    if (3, sj) not in outs_tiles:
                        outs_tiles[(3, sj)] = outsp.tile(
                            [128, D], BF16, tag="outs", name="outs")
                    state["ps"] = ptile
                    row = 512 * 3 + 128 * sj
                    nc.tensor.matmul(state["ps"], oT[1][:, row:row + 128],
                                     wo_sb[:, 1, 512 * do:512 * do + 512],
                                     start=True, stop=True)

                def drain():
                    outt = outs_tiles[(3, sj)]
                    nc.vector.tensor_add(outt[:, 512 * do:512 * do + 512],
                                         state["ps"], outs0[:, 2 * sj + do, :])
                    if do == 1:
                        row = 512 * 3 + 128 * sj
                        nc.sync.dma_start(out=out[row:row + 128, :], in_=outt)
                        del outs_tiles[(3, sj)]
                return [mm, drain]

            # ---------------- drip schedule (greedy backward fill) --------
            # Chains go through ONE serial misc PSUM bank, so they must run
            # contiguously in NEED ORDER (interleaving two chains through the
            # pool would stall the PE queue on the other chain's drain).
            sched = {}

            # prologue chains run before block 0 (positions < 0)
            for i, c in enumerate(qk_chain_chunks("k", 0, 0)):
                sched.setdefault(-10 + i, []).append(c)
            for i, c in enumerate(qk_chain_chunks("q", 0, 0)):
                sched.setdefault(-5 + i, []).append(c)

            chains = []  # (need_by_pos, earliest_pos, chunks)
            for tr in range(1, 4):
                chains.append((8 * tr + 2, 0, qk_chain_chunks("k", 0, tr)))
            for qt in range(1, 4):
                chains.append((32 * qt - 2, 0, qk_chain_chunks("q", 0, qt)))
            # v projections: the p0 halves feed section-0 trail; p1 halves
            # are not needed until the trail crosses into pair-1 (unit 64)
            for st in range(16):
                chains.append((trail_pos(st) - 2, 0, v_chain_chunks(st, 0)))
            for st in range(16):
                chains.append((trail_pos(64 + st) - 4, 0,
                               v_chain_chunks(st, 1)))
            for tr in range(4):
                chains.append((102 + 7 * tr, 0, qk_chain_chunks("k", 1, tr)))
            for qt in range(4):
                chains.append((max(126, 122 + 32 * qt), 0,
                               qk_chain_chunks("q", 1, qt)))
            # outproj qt 0-2: available only after normalize(4+qt)
            for qt in range(3):
                avail = trail_pos(16 * (4 + qt) + 15) + 2
                for sj in range(4):
                    for do in range(2):
                        chains.append((avail + 30, avail,
                                       outproj_chunks(qt, sj, do)))
            # qt=3 fc0 partials: oT[0][:, q3] ready after normalize(p0, q3)
            avail3 = trail_pos(16 * 3 + 15) + 2
            for sj in range(4):
                for do in range(2):
                    chains.append((avail3 + 25, avail3,
                                   outproj3_pre_chunks(sj, do)))

            chains.sort(key=lambda c: c[0])
            cursor = 0
            for need, earliest, chunks in chains:
                start = max(cursor, earliest, need - len(chunks))
                for i, c in enumerate(chunks):
                    sched.setdefault(start + i, []).append(c)
                cursor = start + len(chunks)

            # ---------------- PE warmup (no input deps) -------------------
            # pipelined (rotating psum banks in the sc pool) so it retires in
            # ~2us and never delays the first projection chain
            warm_rhs = const.tile([1, 512], BF16, tag="warm_rhs")
            nc.vector.memset(warm_rhs, 0.0)
            warm_ps = rsp.tile([128, 512], F32, tag="rs", name="warm_ps")
            for j in range(10):
                nc.tensor.matmul(warm_ps[32 * (j % 2):32 * (j % 2) + 1, :],
                                 onesr_sb[0:1, 0:1], warm_rhs,
                                 start=True, stop=True)
            nc.vector.tensor_copy(warm_rhs, warm_ps[0:1, :])

            # ---------------- streaming state -----------------------------
            sc_tiles = {}     # seg -> psum tile
            exa_tiles = {}    # seg -> sbuf tile
            av_t = {}         # section -> av psum tile
            rs_t = {}         # section -> rs psum tile

            def exa_ap(b):
                si, off = BLOCK_SEG[b]
                return exa_tiles[si][:, 512 * off:512 * off + 512]

            def score_mm(b):
                u, h = b // 2, b % 2
                s, kt = u // 16, u % 16
                p, qt = s // 4, s % 4
                si, off = BLOCK_SEG[b]
                if off == 0:
                    sc_tiles[si] = sc2p.tile([128, 1024], F32,
                                             tag="sc2", name="sc")
                sc = sc_tiles[si]
                nc.tensor.matmul(
                    sc[:, 512 * off:512 * off + 512],
                    kp[p][64 * h:64 * h + 64, 128 * kt:128 * kt + 128],
                    qp[p][64 * h:64 * h + 64, 512 * qt:512 * qt + 512],
                    start=True, stop=True)

            def exp_seg(si):
                exa_tiles[si] = exa2p.tile([128, 1024], BF16,
                                           tag="exa2", name="exa")
                nc.scalar.activation(out=exa_tiles[si], in_=sc_tiles[si],
                                     func=EXP, scale=1.0 / np.sqrt(DH))
                del sc_tiles[si]

            def section_start(s):
                av_t[s] = avp.tile([128, 512], F32, tag="av", name="av")
                rs_t[s] = rsp.tile([128, 512], F32, tag="rs", name="rs")
                nc.vector.memset(av_t[s], 0.0)
                nc.vector.memset(rs_t[s], 0.0)

            def trail_pair(u):
                """Trail work for units (u-1, u), u odd: 4 av MMs as two
                col-tiled pairs back-to-back, then the 4-way rowsum quad.
                Same-kind matmuls are batched so the tile-position
                concurrency engages (interleaving kinds serializes them)."""
                s = u // 16
                p = s // 4
                if (u - 1) % 16 == 0:
                    section_start(s)
                av = av_t[s]
                for uu in (u - 1, u):
                    st = uu % 16
                    for h in range(2):
                        nc.tensor.matmul(
                            av[64 * h:64 * h + 64, :],
                            vp[st][:, 128 * p + 64 * h:128 * p + 64 * h + 64],
                            exa_ap(2 * uu + h),
                            start=False, stop=(uu % 16 == 15),
                            tile_position=(0, 64 * h), skip_group_check=True)
                rs = rs_t[s]
                last = (u % 16 == 15)
                if s == 7:
                    # both parities accumulate into rows 0/32 so the tail
                    # normalize needs no cross-partition DMA combine
                    for uu in (u - 1, u):
                        for h in range(2):
                            nc.tensor.matmul(
                                rs[32 * h:32 * h + 1, :], ones_sb,
                                exa_ap(2 * uu + h),
                                start=False, stop=last,
                                tile_position=(0, 32 * h),
                                skip_group_check=True)
                else:
                    for (uu, h) in ((u - 1, 0), (u - 1, 1), (u, 0), (u, 1)):
                        row = 64 * (uu % 2) + 32 * h
                        nc.tensor.matmul(
                            rs[row:row + 1, :], ones_sb, exa_ap(2 * uu + h),
                            start=False, stop=last,
                            tile_position=(0, row), skip_group_check=True)
                if last:
                    normalize(s)

            def normalize(s):
                p, qt = s // 4, s % 4
                av, rs = av_t.pop(s), rs_t.pop(s)
                stage = small.tile([128, 512], F32, tag="stage", name="stage")
                nc.vector.tensor_copy(stage, rs)
                nrm = small.tile([1, 2048], F32, tag="nrm", name="nrm")
                bc = small.tile([128, 1024], F32, tag="bc", name="bc")
                # den layout: [h0_even | h1_even | h0_odd | h1_odd]
                nc.sync.dma_start(out=nrm[0:1, 0:512], in_=stage[0:1, :])
                nc.sync.dma_start(out=nrm[0:1, 512:1024], in_=stage[32:33, :])
                if s != 7:
                    nc.sync.dma_start(out=nrm[0:1, 1024:1536],
                                      in_=stage[64:65, :])
                    nc.sync.dma_start(out=nrm[0:1, 1536:2048],
                                      in_=stage[96:97, :])
                    nc.vector.tensor_add(nrm[0:1, 0:1024], nrm[0:1, 0:1024],
                                         nrm[0:1, 1024:2048])
                rr = nrm[0:1, 1024:2048]
                nc.vector.reciprocal_approx_fast(rr, nrm[0:1, 0:1024])
                nc.gpsimd.partition_broadcast(bc, rr)
                nc.vector.tensor_mul(oT[p][0:64, 512 * qt:512 * qt + 512],
                                     av[0:64, :], bc[0:64, 0:512])
                nc.vector.tensor_mul(oT[p][64:128, 512 * qt:512 * qt + 512],
                                     av[64:128, :], bc[64:128, 512:1024])

            # ---------------- main stream ---------------------------------
            START = min(list(sched.keys()) + [0])
            for pos in range(START, 0):
                for fn in sched.pop(pos, []):
                    fn()

            # score MMs are emitted in 2-unit batches (4 adjacent MMs
            # alternating row halves) so the row-tiled concurrency engages;
            # the two exps follow immediately so ACT is never starved
            for pos in range(LAST_POS + 1):
                if pos < NB and pos % 4 == 0:
                    for b in range(pos, pos + 4):
                        score_mm(b)
                    exp_seg(pos // 2)
                    exp_seg(pos // 2 + 1)
                if pos in TRAIL_AT:
                    trail_pair(TRAIL_AT[pos])
                for fn in sched.pop(pos, []):
                    fn()

            # ---------------- tail: outproj for qt=3 ----------------------
            # rotate psum across the freed score banks for pipelining
            tail_ps = []
            t2a = sc2p.tile([128, 1024], F32, tag="sc2", name="sc_tail")
            t2b = sc2p.tile([128, 1024], F32, tag="sc2", name="sc_tail2")
            for t2 in (t2a, t2b):
                for j in range(2):
                    tail_ps.append(t2[:, 512 * j:512 * j + 512])
            jobs = [(sj, do) for sj in range(4) for do in range(2)]
            pending = []
            for i, (sj, do) in enumerate(jobs):
                mm, drain = outproj3_tail(sj, do, tail_ps[i % len(tail_ps)])
                mm()
                pending.append(drain)
                if len(pending) >= 3:
                    pending.pop(0)()
            for d in pending:
                d()
            # flush any unscheduled chunks (shouldn't happen)
            for pos in sorted(sched):
                for fn in sched[pos]:
                    fn()

    nc.compile()
    return nc


_CACHE = {}


def _get_nc():
    if "nc" not in _CACHE:
        _CACHE["nc"] = build_nc()
    return _CACHE["nc"]


def _prep_inputs(query, key, value, in_proj_w, in_proj_b, out_proj_w):
    bf16 = ml_dtypes.bfloat16
    wq, wk, wv = (in_proj_w[0:D], in_proj_w[D:2 * D], in_proj_w[2 * D:3 * D])
    bq, bk, bv = (in_proj_b[0:D], in_proj_b[D:2 * D], in_proj_b[2 * D:3 * D])

    qT = [np.ascontiguousarray(query[b].T).astype(bf16) for b in range(B)]
    kT = [np.ascontiguousarray(key[b].T).astype(bf16) for b in range(B)]
    vT = [np.ascontiguousarray(value[b].T).astype(bf16) for b in range(B)]

    in_maps = []
    for b in range(B):
        for g in range(GROUPS):
            fs = slice(GF * g, GF * (g + 1))
            in_maps.append({
                "qT": qT[b], "kT": kT[b], "vT": vT[b],
                "wqT": np.ascontiguousarray(wq[fs].T).astype(bf16),
                "wkT": np.ascontiguousarray(wk[fs].T).astype(bf16),
                "wvT": np.ascontiguousarray(wv[fs].T).astype(bf16),
                "woT": np.ascontiguousarray(out_proj_w[:, fs].T).astype(bf16),
                "bq": np.ascontiguousarray(bq[fs]).astype(np.float32),
                "bk": np.ascontiguousarray(bk[fs]).astype(np.float32),
                "bv": np.ascontiguousarray(bv[fs]).astype(bf16),
            })
    return in_maps


def kernel(query, key, value, in_proj_w, in_proj_b, out_proj_w, out_proj_b,
           mask_w, mask_b, _run_kwargs=None):
    query = np.asarray(query, np.float32)
    key = np.asarray(key, np.float32)
    value = np.asarray(value, np.float32)
    in_proj_w = np.asarray(in_proj_w, np.float32)
    in_proj_b = np.asarray(in_proj_b, np.float32)
    out_proj_w = np.asarray(out_proj_w, np.float32)
    out_proj_b = np.asarray(out_proj_b, np.float32)
    mask_w = np.asarray(mask_w, np.float32)
    mask_b = np.asarray(mask_b, np.float32)

    in_maps = _prep_inputs(query, key, value, in_proj_w, in_proj_b, out_proj_w)
    nc = _get_nc()
    for _attempt in range(3):
        res = run_bass_kernel_spmd(nc, in_maps, core_ids=list(range(NCORES)),
                                   **(_run_kwargs or {}))
        parts = [np.asarray(r["out_part"], np.float32) for r in res.results]
        # guard against rare transient device glitches: partial outputs are
        # normally bounded well under 1
        if all(np.isfinite(p).all() and np.abs(p).max() < 100.0 for p in parts):
            break
    mha = np.stack(
        [sum(parts[b * GROUPS + g] for g in range(GROUPS)) for b in range(B)],
        axis=0,
    ) + out_proj_b[None, None, :].astype(np.float32)

    logit = (query[:, -1] @ mask_w.T + mask_b).astype(np.float64)
    m = (1.0 / (1.0 + np.exp(-logit))).astype(np.float32).reshape(B)
    c = np.where(m < 0.1, np.float32(1.0) - m, np.float32(1.0))

    out_full = c[:, None, None, None] * mha[None, :, :, :]
    if _run_kwargs is not None:
        _CACHE["last_results"] = res
    return out_full.astype(np.float32)
